# revision 25
# baseline (speedup 1.0000x reference)
"""AttentionBlock kernel for Trainium2 (8 NeuronCores, batch-sharded).

Per sample b:
    q = Wq @ x + bq            [32, N]
    k = Wk @ x + bk            [32, N]
    v = Wv @ x + bv            [256, N]
    attn = softmax(q^T k)      [N, N] (softmax over keys)
    out = gamma * (v @ attn^T) + x

Transpose-free layout: S^T [keys, queries] is produced directly, the
softmax denominator (a partition-dim sum) comes from ones-matmuls
col-packed 4x via tile_position, and normalization is deferred to the
[256, N] output.  The K=32 logit matmuls are row-packed 4x via
tile_position (q/k replicated to all four 32-partition groups).
Logits run in float32r (1 cycle/row); exp output, PV and denominator
matmuls run in bf16.
"""

from contextlib import ExitStack

import numpy as np

import concourse.bass as bass
import concourse.mybir as mybir
import concourse.tile as tile
from concourse import bacc
from concourse.bass_utils import run_bass_kernel_spmd

B, C, H, W = 8, 256, 64, 64
N = H * W        # 4096
D = 32           # C // 8
NCORES = 8
P = 128
F32 = mybir.dt.float32
F32R = mybir.dt.float32r
BF16 = mybir.dt.bfloat16

NW = 8           # n-chunks of 512 queries
NCH = N // NW    # 512
MP = N // P      # 32 key-chunks of 128
QUAD = 4         # key-chunks per group (row/col packed)
NG = MP // QUAD  # 8 groups


def build_bass():
    nc = bacc.Bacc("TRN2", target_bir_lowering=False, debug=False,
                   enable_asserts=False, num_devices=NCORES)

    x_d = nc.dram_tensor("x", [C, N], F32R, kind="ExternalInput").ap()
    wqT_d = nc.dram_tensor("wqT", [C, D], F32R, kind="ExternalInput").ap()
    wkT_d = nc.dram_tensor("wkT", [C, D], F32R, kind="ExternalInput").ap()
    wvT_d = nc.dram_tensor("wvT", [C, C], F32R, kind="ExternalInput").ap()
    bq_d = nc.dram_tensor("bq", [D, 1], F32, kind="ExternalInput").ap()
    bk_d = nc.dram_tensor("bk", [D, 1], F32, kind="ExternalInput").ap()
    bvb_d = nc.dram_tensor("bvb", [P, C], F32, kind="ExternalInput").ap()
    igam_d = nc.dram_tensor("igam", [P, 1], F32, kind="ExternalInput").ap()
    ones16_d = nc.dram_tensor("ones16", [P, D], BF16, kind="ExternalInput").ap()
    ones32_d = nc.dram_tensor("ones32", [P, P], F32R, kind="ExternalInput").ap()
    out_d = nc.dram_tensor("out", [C, N], F32, kind="ExternalOutput").ap()

    with tile.TileContext(nc) as tc, ExitStack() as ctx:
        const = ctx.enter_context(tc.tile_pool(name="const", bufs=1))
        xp = ctx.enter_context(tc.tile_pool(name="xp", bufs=1))
        qk = ctx.enter_context(tc.tile_pool(name="qk", bufs=1))
        vt = ctx.enter_context(tc.tile_pool(name="vt", bufs=1))
        pt = ctx.enter_context(tc.tile_pool(name="pt", bufs=5))
        op = ctx.enter_context(tc.tile_pool(name="op", bufs=2))
        ps_st = ctx.enter_context(tc.tile_pool(name="ps_st", bufs=2, space="PSUM"))
        ps_out = ctx.enter_context(tc.tile_pool(name="ps_out", bufs=1, space="PSUM"))
        ps_den = ctx.enter_context(tc.tile_pool(name="ps_den", bufs=2, space="PSUM"))

        # ---- load inputs: small weights first, then x chunks in the
        # order the prologue consumes them ----
        wqT_sb = const.tile([P, 2, D], F32R)
        nc.sync.dma_start(out=wqT_sb[:, 0, :], in_=wqT_d[0:P, :])
        nc.sync.dma_start(out=wqT_sb[:, 1, :], in_=wqT_d[P:C, :])
        wkT_sb = const.tile([P, 2, D], F32R)
        nc.sync.dma_start(out=wkT_sb[:, 0, :], in_=wkT_d[0:P, :])
        nc.sync.dma_start(out=wkT_sb[:, 1, :], in_=wkT_d[P:C, :])
        wvT_sb = const.tile([P, 2, C], F32R)
        nc.sync.dma_start(out=wvT_sb[:, 0, :], in_=wvT_d[0:P, :])
        nc.sync.dma_start(out=wvT_sb[:, 1, :], in_=wvT_d[P:C, :])
        bq_sb = const.tile([D, 1], F32)
        nc.sync.dma_start(out=bq_sb, in_=bq_d)
        bk_sb = const.tile([D, 1], F32)
        nc.sync.dma_start(out=bk_sb, in_=bk_d)
        bvb_sb = const.tile([P, C], F32)
        nc.sync.dma_start(out=bvb_sb, in_=bvb_d)
        igam_sb = const.tile([P, 1], F32)
        nc.sync.dma_start(out=igam_sb, in_=igam_d)
        ones16_sb = const.tile([P, D], BF16)
        nc.sync.dma_start(out=ones16_sb, in_=ones16_d)
        ones32_sb = const.tile([P, P], F32R)      # value 1/32
        nc.sync.dma_start(out=ones32_sb, in_=ones32_d)

        x_sb = xp.tile([P, 2, N], F32R)           # [128, c-half, 4096]
        QP = NCH // 4
        for j in range(NW):
            for qq in range(4):
                for ci in range(2):
                    sl = slice(j * NCH + qq * QP, j * NCH + (qq + 1) * QP)
                    nc.sync.dma_start(out=x_sb[:, ci, sl],
                                      in_=x_d[ci * P:(ci + 1) * P, sl])

        # ---- prologue ----
        # q replicated to 4 partition groups; k packed [group j][g, 128]
        q_pack = qk.tile([P, N], BF16)
        k_sb = qk.tile([D, N], BF16)
        k_pack = qk.tile([P, NG, P], BF16)
        vT16_sb = vt.tile([P, MP, C], BF16)       # [128, m-chunk, 256]

        _pro = [(ps_st, "stq"), (ps_out, "outq"), (ps_den, "den")]

        def pro_ps(idx, shape, tag_pair):
            pool, tg = _pro[idx % 3]
            return pool.tile(shape, F32, name=f"pro_{tag_pair}_{idx}", tag=tg)

        for j in range(NW):
            sl = slice(j * NCH, (j + 1) * NCH)
            ps_q = pro_ps(j, [D, NCH], "q")
            for ci in range(2):
                nc.tensor.matmul(ps_q, lhsT=wqT_sb[:, ci, :],
                                 rhs=x_sb[:, ci, sl],
                                 start=(ci == 0), stop=(ci == 1))
            nc.vector.tensor_scalar_add(out=q_pack[0:D, sl], in0=ps_q,
                                        scalar1=bq_sb)
            ps_k = pro_ps(j + 1, [D, NCH], "k")
            for ci in range(2):
                nc.tensor.matmul(ps_k, lhsT=wkT_sb[:, ci, :],
                                 rhs=x_sb[:, ci, sl],
                                 start=(ci == 0), stop=(ci == 1))
            nc.vector.tensor_scalar_add(out=k_sb[:, sl], in0=ps_k,
                                        scalar1=bk_sb)
            # replicate q chunk to partition groups 1..3; scatter k chunk
            # (key-chunks 4j..4j+3) into k_pack quad column j
            for jj in range(1, 4):
                nc.sync.dma_start(out=q_pack[D * jj:D * (jj + 1), sl],
                                  in_=q_pack[0:D, sl])
            k_chunk = k_sb[:, sl].rearrange("p (jj c) -> p jj c",
                                            jj=QUAD, c=P)
            for jj in range(4):
                nc.sync.dma_start(out=k_pack[D * jj:D * (jj + 1), j, :],
                                  in_=k_chunk[:, jj, :])

        for m in range(MP):
            msl = slice(m * P, (m + 1) * P)
            ps_v = pro_ps(m, [P, C], "v")
            for ci in range(2):
                nc.tensor.matmul(ps_v, lhsT=x_sb[:, ci, msl],
                                 rhs=wvT_sb[:, ci, :],
                                 start=(ci == 0), stop=(ci == 1))
            nc.vector.tensor_add(out=vT16_sb[:, m, :], in0=ps_v, in1=bvb_sb)

        # ---- main attention loop ----
        # Software-pipelined per chunk: quad g's S^T+exp issue before
        # quad g-1's PV/den so the PE never waits on the ScalarE exp.
        for n in range(NW):
            nsl = slice(n * NCH, (n + 1) * NCH)
            out_ps = ps_out.tile([P, 2, NCH], F32, tag="outq")   # 2 banks
            den_ps = ps_den.tile([P, NCH], F32, tag="den")       # 1 bank
            pend = {}
            for g in range(NG + 1):
                if g < NG:
                    st_a = ps_st.tile([P, 2, NCH], F32, tag="stq")
                    st_b = ps_st.tile([P, 2, NCH], F32, tag="stq")
                    for j in range(QUAD):
                        dst = st_a if j < 2 else st_b
                        nc.tensor.matmul(dst[:, j % 2, :],
                                         lhsT=k_pack[D * j:D * (j + 1), g, :],
                                         rhs=q_pack[D * j:D * (j + 1), nsl],
                                         start=True, stop=True,
                                         tile_position=(D * j, 0))
                    p_a = pt.tile([P, 2, NCH], BF16)
                    nc.scalar.activation(out=p_a, in_=st_a,
                                         func=mybir.ActivationFunctionType.Exp)
                    p_b = pt.tile([P, 2, NCH], BF16)
                    nc.scalar.activation(out=p_b, in_=st_b,
                                         func=mybir.ActivationFunctionType.Exp)
                    pend[g] = (p_a, p_b)
                if g > 0:
                    gg = g - 1
                    p_a, p_b = pend.pop(gg)
                    first = (gg == 0)
                    last = (gg == NG - 1)
                    for j in range(QUAD):
                        m = gg * QUAD + j
                        prhs = (p_a if j < 2 else p_b)[:, j % 2, :]
                        nc.tensor.matmul(out_ps[:, 0, :],
                                         lhsT=vT16_sb[:, m, 0:P], rhs=prhs,
                                         start=(first and j == 0),
                                         stop=(last and j == QUAD - 1))
                        nc.tensor.matmul(out_ps[:, 1, :],
                                         lhsT=vT16_sb[:, m, P:C], rhs=prhs,
                                         start=(first and j == 0),
                                         stop=(last and j == QUAD - 1))
                    for j in range(QUAD):
                        prhs = (p_a if j < 2 else p_b)[:, j % 2, :]
                        nc.tensor.matmul(den_ps[D * j:D * (j + 1), :],
                                         lhsT=ones16_sb, rhs=prhs,
                                         start=first, stop=last,
                                         tile_position=(0, D * j))
            # den finish: den_sb = den_ps/|gamma| (copy+scale on DVE),
            # broadcast-sum via ones32 (1/32) matmul, rd = |gamma|/den.
            # sign(gamma) is folded into Wv/bv host-side.
            den_sb = op.tile([P, NCH], F32R)
            nc.vector.tensor_scalar_mul(out=den_sb, in0=den_ps,
                                        scalar1=igam_sb)
            den_b = ps_den.tile([P, NCH], F32, tag="den")
            nc.tensor.matmul(den_b, lhsT=ones32_sb, rhs=den_sb,
                             start=True, stop=True)
            rd_sb = op.tile([P, NCH], F32)
            nc.vector.reciprocal_approx_fast(out=rd_sb, in_=den_b)
            # normalize: out = rd * num + x
            out_sb = op.tile([P, 2, NCH], F32)
            for hh in range(2):
                nc.vector.tensor_mul(out=out_sb[:, hh, :],
                                     in0=out_ps[:, hh, :], in1=rd_sb)
            for hh in range(2):
                nc.vector.tensor_add(out=out_sb[:, hh, :],
                                     in0=out_sb[:, hh, :],
                                     in1=x_sb[:, hh, nsl].bitcast(F32))
                nc.sync.dma_start(out=out_d[hh * P:(hh + 1) * P, nsl],
                                  in_=out_sb[:, hh, :])
    nc.compile()
    return nc


_NC_CACHE = None


def _get_nc():
    global _NC_CACHE
    if _NC_CACHE is None:
        _NC_CACHE = build_bass()
    return _NC_CACHE


def _in_maps(inputs):
    import ml_dtypes
    x = np.ascontiguousarray(np.asarray(inputs["x"], dtype=np.float32))
    wqT = np.ascontiguousarray(np.asarray(inputs["Wq"], np.float32).T)
    wkT = np.ascontiguousarray(np.asarray(inputs["Wk"], np.float32).T)
    wvT = np.ascontiguousarray(np.asarray(inputs["Wv"], np.float32).T)
    bq = np.asarray(inputs["bq"], np.float32).reshape(D, 1).copy()
    bk = np.asarray(inputs["bk"], np.float32).reshape(D, 1).copy()
    gamma = float(np.asarray(inputs["gamma"], np.float32).reshape(()))
    sg = 1.0 if gamma >= 0 else -1.0
    wvT = np.ascontiguousarray(wvT * sg)
    bvb = np.ascontiguousarray(
        sg * np.broadcast_to(np.asarray(inputs["bv"], np.float32)[None, :],
                             (P, C)))
    igam = np.full((P, 1), 1.0 / abs(gamma), np.float32)
    ones16 = np.ones((P, D), np.float32).astype(ml_dtypes.bfloat16)
    ones32 = np.full((P, P), 1.0 / 32.0, np.float32)
    maps = []
    for b in range(NCORES):
        maps.append({
            "x": np.ascontiguousarray(x[b].reshape(C, N)),
            "wqT": wqT, "wkT": wkT, "wvT": wvT,
            "bq": bq, "bk": bk, "bvb": bvb, "igam": igam,
            "ones16": ones16, "ones32": ones32,
        })
    return maps


def _run(inputs, **kw):
    nc = _get_nc()
    res = run_bass_kernel_spmd(nc, _in_maps(inputs), core_ids=list(range(NCORES)),
                               **kw)
    outs = [res.results[b]["out"].reshape(C, H, W) for b in range(NCORES)]
    return np.stack(outs, axis=0).astype(np.float32), res


def kernel(**inputs) -> np.ndarray:
    out, _ = _run(inputs)
    return out
